# revision 18
# baseline (speedup 1.0000x reference)
"""Trainium2 Bass kernel for nn_CausalShapedAttention (B=2, T=2048, C=1024, H=16).

Sharding: 8 cores = 2 batches x 4 head-groups (4 heads each). Each core
computes qkv for its heads, causal shaped attention, and a partial output
projection; the host sums the 4 partial projections per batch and adds b_proj.

Math per head (d=64):
    scores = q @ k.T / 8           (only causal lower triangle computed)
    att    = exp(scores) / rowsum(exp(scores))     # no max-subtraction:
                                                   # |scores| <~ 4, exp safe
    y      = beta * att @ v + alpha * v - gamma * cummean(v)

On-device layouts (per core):
    xT  [C, T]    qT/kT [256, T] (bf16 for PE)     v [T, 256] (f32r)
    scoresT strips [s-block 128, t >= s]  -> exp (bf16) -> att@v accumulates
    yT_unnorm in PSUM [65, T] via ones-augmented beta*v (row 64 = softmax denom)
    cummean via PE transpose of v blocks + DVE prefix scan
    proj: outT [C, T] = wpT.T @ yT   (partial; host reduces over groups)
"""

import numpy as np
import ml_dtypes

import concourse.bass as bass
import concourse.mybir as mybir
import concourse.tile as tile
from concourse import bacc, bass_utils
from concourse.bass import ds, ts

B, T, C, H = 2, 2048, 1024, 16
NCORES = 8
G = 4                 # head-groups per batch
HPG = H // G          # heads per group = 4
D = C // H            # 64
E = HPG * D           # 256

f32 = mybir.dt.float32
f32r = mybir.dt.float32r
bf16 = mybir.dt.bfloat16
AFT = mybir.ActivationFunctionType
ALU = mybir.AluOpType

KT = C // 128         # 8 k-tiles over the embedding dim
TT = T // 128         # 16 t-tiles
NB = T // 512         # 4 psum banks over T


def build_bass():
    nc = bacc.Bacc("TRN2", target_bir_lowering=False, debug=False)

    # ---- DRAM I/O ----
    xT_d = nc.dram_tensor("xT", [C, T], f32r, kind="ExternalInput").ap()
    wq_d = nc.dram_tensor("wq", [C, E], f32r, kind="ExternalInput").ap()
    wk_d = nc.dram_tensor("wk", [C, E], f32r, kind="ExternalInput").ap()
    wv_d = nc.dram_tensor("wv", [C, E], f32r, kind="ExternalInput").ap()
    wp_d = nc.dram_tensor("wp", [E, C], f32r, kind="ExternalInput").ap()
    bq_d = nc.dram_tensor("bq", [128, 2], f32, kind="ExternalInput").ap()
    bk_d = nc.dram_tensor("bk", [128, 2], f32, kind="ExternalInput").ap()
    bv_d = nc.dram_tensor("bv", [1, E], f32, kind="ExternalInput").ap()
    agt_d = nc.dram_tensor("agt", [1, T], f32, kind="ExternalInput").ap()
    nalpha_d = nc.dram_tensor("nalpha", [128, 1], f32, kind="ExternalInput").ap()
    beta_d = nc.dram_tensor("betac", [128, 1], f32, kind="ExternalInput").ap()
    ident_d = nc.dram_tensor("ident", [128, 128], f32r, kind="ExternalInput").ap()
    slt_d = nc.dram_tensor("slt", [128, 128], bf16, kind="ExternalInput").ap()
    negid_d = nc.dram_tensor("negid", [128, 128], bf16, kind="ExternalInput").ap()
    outT_d = nc.dram_tensor("outT", [C, T], f32, kind="ExternalOutput").ap()

    bounce_d = [
        nc.dram_tensor(f"bounce{h}", [1, T], f32, kind="Internal").ap()
        for h in range(HPG)
    ]

    with tile.TileContext(nc) as tc:
        import contextlib

        with contextlib.ExitStack() as ctx:
            p_big = ctx.enter_context(tc.tile_pool(name="p_big", bufs=1))
            p_w = ctx.enter_context(tc.tile_pool(name="p_w", bufs=1))
            p_wp = ctx.enter_context(tc.tile_pool(name="p_wp", bufs=1))
            p_const = ctx.enter_context(tc.tile_pool(name="p_const", bufs=1))
            p_qk = ctx.enter_context(tc.tile_pool(name="p_qk", bufs=1))
            p_v = ctx.enter_context(tc.tile_pool(name="p_v", bufs=1))
            p_exp = ctx.enter_context(tc.tile_pool(name="p_exp", bufs=3))
            p_y = ctx.enter_context(tc.tile_pool(name="p_y", bufs=1))
            p_pb = ctx.enter_context(tc.tile_pool(name="p_pb", bufs=1))
            ps_mm = ctx.enter_context(tc.tile_pool(name="ps_mm", bufs=2, space="PSUM"))
            ps_y = ctx.enter_context(tc.tile_pool(name="ps_y", bufs=1, space="PSUM"))

            # ---- x streams in first (critical path), then weights ----
            x_sbT = p_big.tile([128, KT, T], f32r, tag="big")
            for k in range(KT):
                nc.sync.dma_start(x_sbT[:, k, :], xT_d[ts(k, 128), :])

            wq_sb = p_w.tile([128, KT, E], f32r, tag="wq")
            nc.sync.dma_start(wq_sb, wq_d.rearrange("(k p) j -> p k j", p=128))
            wk_sb = p_w.tile([128, KT, E], f32r, tag="wk")
            nc.sync.dma_start(wk_sb, wk_d.rearrange("(k p) j -> p k j", p=128))

            bq_sb = p_const.tile([128, 2], f32)
            nc.sync.dma_start(bq_sb, bq_d)
            bk_sb = p_const.tile([128, 2], f32)
            nc.sync.dma_start(bk_sb, bk_d)
            nalpha_sb = p_const.tile([128, 1], f32)
            nc.sync.dma_start(nalpha_sb, nalpha_d)
            beta_sb = p_const.tile([128, 1], f32)
            nc.sync.dma_start(beta_sb, beta_d)
            ident_sb = p_const.tile([128, 128], f32r)
            nc.sync.dma_start(ident_sb, ident_d)
            slt_sb = p_const.tile([128, 128], bf16)
            nc.sync.dma_start(slt_sb, slt_d)
            negid_sb = p_const.tile([128, 128], bf16)
            nc.sync.dma_start(negid_sb, negid_d)
            bv_bc = p_const.tile([128, E], f32)
            nc.sync.dma_start(
                bv_bc,
                bass.AP(tensor=bv_d.tensor, offset=bv_d.offset,
                        ap=[[0, 128]] + bv_d.ap[1:]),
            )
            agt_bc = p_const.tile([64, T], f32)
            nc.sync.dma_start(
                agt_bc,
                bass.AP(tensor=agt_d.tensor, offset=agt_d.offset,
                        ap=[[0, 64]] + agt_d.ap[1:]),
            )

            wv_sb = p_w.tile([128, KT, E], f32r, tag="wv")
            nc.sync.dma_start(wv_sb, wv_d.rearrange("(k p) j -> p k j", p=128))
            wp_sb = p_wp.tile([128, 2, C], f32r)
            nc.sync.dma_start(wp_sb, wp_d.rearrange("(m p) o -> p m o", p=128))

            # ---- qkv ----
            # m=0 (heads 0/1) first, then v, then m=1 — the m=1 matmuls act
            # as PE filler while head 0/1's ACT-bound softmax runs
            qT_sb = p_qk.tile([128, HPG, T], bf16, tag="qT")
            kT_sb = p_qk.tile([128, HPG, T], bf16, tag="kT")
            nc.vector.memset(qT_sb[64:128, :, :], 0.0)
            nc.vector.memset(kT_sb[64:128, :, :], 0.0)

            def emit_qk(m):
                for w_sb, b_sb, dst in (
                    (wq_sb, bq_sb, qT_sb),
                    (wk_sb, bk_sb, kT_sb),
                ):
                    for n in range(NB):
                        ps_q = ps_mm.tile([128, 512], f32, tag="mm512")
                        for k in range(KT):
                            nc.tensor.matmul(
                                ps_q,
                                w_sb[:, k, ts(m, 128)],
                                x_sbT[:, k, ts(n, 512)],
                                start=(k == 0),
                                stop=(k == KT - 1),
                            )
                        for hh in range(2):
                            nc.vector.tensor_scalar_add(
                                dst[0:64, 2 * m + hh, ts(n, 512)],
                                ps_q[64 * hh : 64 * hh + 64, :],
                                b_sb[64 * hh : 64 * hh + 64, m : m + 1],
                            )

            emit_qk(0)

            v_sb = p_v.tile([128, TT, E], f32r, tag="vsb")
            v_bf = p_v.tile([128, TT, HPG * 65], bf16, tag="vbf")
            ones_ap = v_bf.rearrange("p i (h e) -> p i h e", h=HPG)[:, :, :, 64:65]
            nc.vector.memset(ones_ap, 1.0)
            for i in range(TT):
                ps_v = ps_mm.tile([128, 512], f32, tag="mm512")
                for k in range(KT):
                    nc.tensor.matmul(
                        ps_v[:, :E],
                        x_sbT[:, k, ts(i, 128)],
                        wv_sb[:, k, :],
                        start=(k == 0),
                        stop=(k == KT - 1),
                    )
                nc.vector.tensor_add(v_sb[:, i, :], ps_v[:, :E], bv_bc)
                # beta pre-scaled into the attv weights (ones col stays 1 so
                # psum row 64 remains the raw softmax denominator)
                nc.vector.tensor_scalar_mul(
                    v_bf[:, i, :].rearrange("p (h e) -> p h e", h=HPG)[:, :, 0:64],
                    v_sb[:, i, :],
                    beta_sb,
                )

            emit_qk(1)

            # ---- attention heads ----
            yT_all = p_y.tile([128, 2, T], f32r)
            for h in range(HPG):
                po = 64 * (h % 2)
                m = h // 2

                # cumsum of v rows (for the MC + Id terms), ahead of the
                # attention loop so PE transposes interleave with matmuls
                cum_sb = p_w.tile([64, T], f32, tag="wk")
                for i in range(TT):
                    ps_tr = ps_mm.tile([64, 128], f32r, tag="mm512")
                    nc.tensor.transpose(
                        ps_tr, v_sb[:, i, ds(64 * h, 64)], ident_sb
                    )
                    init = 0.0 if i == 0 else cum_sb[:, 128 * i - 1 : 128 * i]
                    nc.vector.tensor_tensor_scan(
                        cum_sb[:, ts(i, 128)], ps_tr.bitcast(f32), agt_bc[:, 0:128],
                        init, ALU.add, ALU.bypass,
                    )
                # va = cum*(alpha - gamma/(t+1)) - alpha*cum[t-1], computed
                # up front so cum_sb's slot frees before the next head
                # (va[0] = cum[0]*agt[0] already correct: cum[-1] := 0)
                va = p_pb.tile([64, T], f32, tag="t2")
                nc.vector.tensor_mul(va, cum_sb, agt_bc)
                nc.vector.scalar_tensor_tensor(
                    va[:, 1:T], cum_sb[:, 0 : T - 1], nalpha_sb[0:64, :],
                    va[:, 1:T], ALU.mult, ALU.add,
                )

                ps_yT = ps_y.tile([128, T], f32, tag="psy")
                for j in range(TT):
                    exp_sb = p_exp.tile([128, T], bf16, tag="exp")
                    for qb in range(j // 8, 2):
                        qlo = max(1024 * qb, 128 * j)
                        qhi = 1024 * (qb + 1)
                        qw = qhi - qlo
                        ps_s = ps_mm.tile([128, 1024], f32, tag="mm512")
                        # score matmuls per 512 psum bank within the chunk
                        for sb_i, lo in enumerate(range(qlo, qhi, 512)):
                            w = min(512, qhi - lo)
                            diag = qb == j // 8 and sb_i == 0
                            nc.tensor.matmul(
                                ps_s[:, ds(lo - qlo, w)],
                                kT_sb[:, h, ts(j, 128)],
                                qT_sb[:, h, ds(lo, w)],
                                start=True,
                                stop=not (diag and True),
                            )
                            if diag:
                                # causal mask fused into the matmul: add
                                # -1e30 * [s > t] so exp underflows to zero
                                # above the diagonal
                                nc.tensor.matmul(
                                    ps_s[:, ds(128 * j - qlo, 128)],
                                    slt_sb, negid_sb,
                                    start=False, stop=True,
                                )
                        nc.scalar.activation(
                            exp_sb[:, ds(qlo - 128 * j, qw)], ps_s[:, :qw],
                            AFT.Exp, scale=0.125,
                        )
                        for cb in range(qlo // 512, qhi // 512):
                            lo = max(512 * cb, qlo)
                            w = 512 * (cb + 1) - lo
                            nc.tensor.matmul(
                                ps_yT[:65, ds(lo, w)],
                                v_bf[:, j, ds(65 * h, 65)],
                                exp_sb[:, ds(lo - 128 * j, w)],
                                start=(j == 0),
                                stop=(j == 4 * cb + 3),
                            )

                # free ps_yT quickly: pull the raw numerator and denominator
                # out of PSUM first so the next head's att@v can start.
                # 1/den via ACT Ln -> Exp(-x) (custom-DVE reciprocal ops are
                # broken on this toolchain; plain DVE reciprocal is ~6.5ns/elem)
                rden = p_pb.tile([1, T], f32, tag="rden")
                nc.scalar.activation(rden, ps_yT[64:65, :], AFT.Ln)
                t1 = p_w.tile([64, T], f32, tag="wv")
                for cb in range(NB):
                    nc.vector.tensor_copy(
                        t1[:, ts(cb, 512)], ps_yT[0:64, ts(cb, 512)]
                    )
                nc.scalar.activation(rden, rden, AFT.Exp, scale=-1.0)

                nc.sync.dma_start(bounce_d[h], rden)
                rb_bc = p_w.tile([64, T], f32, tag="wq")
                nc.sync.dma_start(
                    rb_bc,
                    bass.AP(tensor=bounce_d[h].tensor, offset=bounce_d[h].offset,
                            ap=[[0, 64]] + bounce_d[h].ap[1:]),
                )

                # yT = num*beta/den + va   (chunked so proj can start early)
                ydst = yT_all[po : po + 64, m, :]
                for cb in range(NB):
                    sl = ts(cb, 512)
                    nc.vector.tensor_mul(t1[:, sl], t1[:, sl], rb_bc[:, sl])
                    nc.vector.tensor_add(ydst[:, sl], t1[:, sl], va[:, sl])

            # ---- output projection (partial over this core's heads) ----
            out_sb = p_big.tile([128, KT, T], f32, tag="big")
            for n in range(NB):
                for o in range(KT):
                    ps_p = ps_mm.tile([128, 512], f32, tag="mm512")
                    for m in range(2):
                        nc.tensor.matmul(
                            ps_p,
                            wp_sb[:, m, ts(o, 128)],
                            yT_all[:, m, ts(n, 512)],
                            start=(m == 0),
                            stop=(m == 1),
                        )
                    nc.vector.tensor_copy(out_sb[:, o, ts(n, 512)], ps_p)
            for o in range(KT):
                nc.sync.dma_start(outT_d[ts(o, 128), :], out_sb[:, o, :])

    nc.compile()
    return nc


_NC_CACHE = None


def _get_nc():
    global _NC_CACHE
    if _NC_CACHE is None:
        _NC_CACHE = build_bass()
    return _NC_CACHE


def make_in_maps(x, W_attn, b_attn, W_proj, b_proj, alpha, beta, gamma):
    x = np.asarray(x, dtype=np.float32)
    W_attn = np.asarray(W_attn, dtype=np.float32)
    b_attn = np.asarray(b_attn, dtype=np.float32)
    W_proj = np.asarray(W_proj, dtype=np.float32)
    alpha = float(np.asarray(alpha))
    beta = float(np.asarray(beta))
    gamma = float(np.asarray(gamma))

    ident = np.eye(128, dtype=np.float32)
    slt = np.triu(np.ones((128, 128), dtype=np.float32), 1).astype(ml_dtypes.bfloat16)
    negid = (np.eye(128, dtype=np.float32) * -1e30).astype(ml_dtypes.bfloat16)
    agt = (alpha - gamma / np.arange(1, T + 1, dtype=np.float32)).reshape(1, T)
    nalpha = np.full((128, 1), -alpha, dtype=np.float32)
    betac = np.full((128, 1), beta, dtype=np.float32)

    in_maps = []
    for core in range(NCORES):
        b, g = divmod(core, G)
        sl = slice(E * g, E * (g + 1))
        in_maps.append({
            "xT": np.ascontiguousarray(x[b].T),
            "wq": np.ascontiguousarray(W_attn[sl, :].T),
            "wk": np.ascontiguousarray(W_attn[C:][sl, :].T),
            "wv": np.ascontiguousarray(W_attn[2 * C:][sl, :].T),
            "wp": np.ascontiguousarray(W_proj[:, sl].T),
            "bq": np.ascontiguousarray(b_attn[sl].reshape(2, 128).T),
            "bk": np.ascontiguousarray(b_attn[C:][sl].reshape(2, 128).T),
            "bv": np.ascontiguousarray(b_attn[2 * C:][sl].reshape(1, E)),
            "agt": agt,
            "nalpha": nalpha,
            "betac": betac,
            "ident": ident,
            "slt": slt,
            "negid": negid,
        })
    return in_maps


def _assemble(results, b_proj):
    b_proj = np.asarray(b_proj, dtype=np.float32)
    out = np.empty((B, T, C), dtype=np.float32)
    for b in range(B):
        acc = results[G * b]["outT"].copy()
        for g in range(1, G):
            acc += results[G * b + g]["outT"]
        out[b] = acc.T + b_proj
    return out


def kernel(x, W_attn, b_attn, W_proj, b_proj, alpha, beta, gamma):
    nc = _get_nc()
    in_maps = make_in_maps(x, W_attn, b_attn, W_proj, b_proj, alpha, beta, gamma)
    res = bass_utils.run_bass_kernel_spmd(nc, in_maps, core_ids=list(range(NCORES)))
    return _assemble(res.results, b_proj)


def run_profiled(inputs, trace_cores=None):
    """Run with NTFF hardware profiling; returns (output, BassKernelResults)."""
    nc = _get_nc()
    in_maps = make_in_maps(**inputs)
    res = bass_utils.run_bass_kernel_spmd(
        nc, in_maps, core_ids=list(range(NCORES)), trace=True,
        trace_cores=trace_cores,
    )
    return _assemble(res.results, inputs["b_proj"]), res


# revision 19
# speedup vs baseline: 1.0665x; 1.0665x over previous
"""Trainium2 Bass kernel for nn_CausalShapedAttention (B=2, T=2048, C=1024, H=16).

Sharding: 8 cores = 2 batches x 4 head-groups (4 heads each). Each core
computes qkv for its heads, causal shaped attention, and a partial output
projection; the host sums the 4 partial projections per batch and adds b_proj.

Math per head (d=64):
    scores = q @ k.T / 8           (only causal lower triangle computed)
    att    = exp(scores) / rowsum(exp(scores))     # no max-subtraction:
                                                   # |scores| <~ 4, exp safe
    y      = beta * att @ v + alpha * v - gamma * cummean(v)

On-device layouts (per core):
    xT  [C, T]    qT/kT [256, T] (bf16 for PE)     v [T, 256] (f32r)
    scoresT strips [s-block 128, t >= s]  -> exp (bf16) -> att@v accumulates
    yT_unnorm in PSUM [65, T] via ones-augmented beta*v (row 64 = softmax denom)
    cummean via PE transpose of v blocks + DVE prefix scan
    proj: outT [C, T] = wpT.T @ yT   (partial; host reduces over groups)
"""

import numpy as np
import ml_dtypes

import concourse.bass as bass
import concourse.mybir as mybir
import concourse.tile as tile
from concourse import bacc, bass_utils
from concourse.bass import ds, ts

B, T, C, H = 2, 2048, 1024, 16
NCORES = 8
G = 4                 # head-groups per batch
HPG = H // G          # heads per group = 4
D = C // H            # 64
E = HPG * D           # 256

f32 = mybir.dt.float32
f32r = mybir.dt.float32r
bf16 = mybir.dt.bfloat16
AFT = mybir.ActivationFunctionType
ALU = mybir.AluOpType

KT = C // 128         # 8 k-tiles over the embedding dim
TT = T // 128         # 16 t-tiles
NB = T // 512         # 4 psum banks over T


def build_bass():
    nc = bacc.Bacc("TRN2", target_bir_lowering=False, debug=False)

    # ---- DRAM I/O ----
    xT_d = nc.dram_tensor("xT", [C, T], f32r, kind="ExternalInput").ap()
    wq_d = nc.dram_tensor("wq", [C, E], f32r, kind="ExternalInput").ap()
    wk_d = nc.dram_tensor("wk", [C, E], f32r, kind="ExternalInput").ap()
    wv_d = nc.dram_tensor("wv", [C, E], f32r, kind="ExternalInput").ap()
    wp_d = nc.dram_tensor("wp", [E, C], f32r, kind="ExternalInput").ap()
    bq_d = nc.dram_tensor("bq", [128, 2], f32, kind="ExternalInput").ap()
    bk_d = nc.dram_tensor("bk", [128, 2], f32, kind="ExternalInput").ap()
    bv_d = nc.dram_tensor("bv", [1, E], f32, kind="ExternalInput").ap()
    agt_d = nc.dram_tensor("agt", [1, T], f32, kind="ExternalInput").ap()
    nalpha_d = nc.dram_tensor("nalpha", [128, 1], f32, kind="ExternalInput").ap()
    beta_d = nc.dram_tensor("betac", [128, 1], f32, kind="ExternalInput").ap()
    ident_d = nc.dram_tensor("ident", [128, 128], f32r, kind="ExternalInput").ap()
    slt_d = nc.dram_tensor("slt", [128, 128], bf16, kind="ExternalInput").ap()
    negid_d = nc.dram_tensor("negid", [128, 128], bf16, kind="ExternalInput").ap()
    outT_d = nc.dram_tensor("outT", [C, T], f32, kind="ExternalOutput").ap()

    bounce_d = [
        nc.dram_tensor(f"bounce{h}", [1, T], f32, kind="Internal").ap()
        for h in range(HPG)
    ]

    with tile.TileContext(nc) as tc:
        import contextlib

        with contextlib.ExitStack() as ctx:
            p_big = ctx.enter_context(tc.tile_pool(name="p_big", bufs=1))
            p_w = ctx.enter_context(tc.tile_pool(name="p_w", bufs=1))
            p_wp = ctx.enter_context(tc.tile_pool(name="p_wp", bufs=1))
            p_const = ctx.enter_context(tc.tile_pool(name="p_const", bufs=1))
            p_qk = ctx.enter_context(tc.tile_pool(name="p_qk", bufs=1))
            p_v = ctx.enter_context(tc.tile_pool(name="p_v", bufs=1))
            p_exp = ctx.enter_context(tc.tile_pool(name="p_exp", bufs=3))
            p_y = ctx.enter_context(tc.tile_pool(name="p_y", bufs=1))
            p_pb = ctx.enter_context(tc.tile_pool(name="p_pb", bufs=1))
            ps_mm = ctx.enter_context(tc.tile_pool(name="ps_mm", bufs=2, space="PSUM"))
            ps_y = ctx.enter_context(tc.tile_pool(name="ps_y", bufs=1, space="PSUM"))
            ps_t = ctx.enter_context(tc.tile_pool(name="ps_t", bufs=2, space="PSUM"))

            # ---- x streams in first (critical path), then weights ----
            x_sbT = p_big.tile([128, KT, T], f32r, tag="big")
            for k in range(KT):
                nc.sync.dma_start(x_sbT[:, k, :], xT_d[ts(k, 128), :])

            wq_sb = p_w.tile([128, KT, E], f32r, tag="wq")
            nc.sync.dma_start(wq_sb, wq_d.rearrange("(k p) j -> p k j", p=128))
            wk_sb = p_w.tile([128, KT, E], f32r, tag="wk")
            nc.sync.dma_start(wk_sb, wk_d.rearrange("(k p) j -> p k j", p=128))

            bq_sb = p_const.tile([128, 2], f32)
            nc.sync.dma_start(bq_sb, bq_d)
            bk_sb = p_const.tile([128, 2], f32)
            nc.sync.dma_start(bk_sb, bk_d)
            nalpha_sb = p_const.tile([128, 1], f32)
            nc.sync.dma_start(nalpha_sb, nalpha_d)
            beta_sb = p_const.tile([128, 1], f32)
            nc.sync.dma_start(beta_sb, beta_d)
            ident_sb = p_const.tile([128, 128], f32r)
            nc.sync.dma_start(ident_sb, ident_d)
            slt_sb = p_const.tile([128, 128], bf16)
            nc.sync.dma_start(slt_sb, slt_d)
            negid_sb = p_const.tile([128, 128], bf16)
            nc.sync.dma_start(negid_sb, negid_d)
            bv_bc = p_const.tile([128, E], f32)
            nc.sync.dma_start(
                bv_bc,
                bass.AP(tensor=bv_d.tensor, offset=bv_d.offset,
                        ap=[[0, 128]] + bv_d.ap[1:]),
            )
            agt_bc = p_const.tile([64, T], f32)
            nc.sync.dma_start(
                agt_bc,
                bass.AP(tensor=agt_d.tensor, offset=agt_d.offset,
                        ap=[[0, 64]] + agt_d.ap[1:]),
            )

            wv_sb = p_w.tile([128, KT, E], f32r, tag="wv")
            nc.sync.dma_start(wv_sb, wv_d.rearrange("(k p) j -> p k j", p=128))
            wp_sb = p_wp.tile([128, 2, C], f32r)
            nc.sync.dma_start(wp_sb, wp_d.rearrange("(m p) o -> p m o", p=128))

            # ---- qkv ----
            # m=0 (heads 0/1) first, then v, then m=1 — the m=1 matmuls act
            # as PE filler while head 0/1's ACT-bound softmax runs
            qT_sb = p_qk.tile([128, HPG, T], bf16, tag="qT")
            kT_sb = p_qk.tile([128, HPG, T], bf16, tag="kT")
            nc.vector.memset(qT_sb[64:128, :, :], 0.0)
            nc.vector.memset(kT_sb[64:128, :, :], 0.0)

            def emit_qk(m):
                for w_sb, b_sb, dst in (
                    (wq_sb, bq_sb, qT_sb),
                    (wk_sb, bk_sb, kT_sb),
                ):
                    for n in range(NB):
                        ps_q = ps_mm.tile([128, 512], f32, tag="mm512")
                        for k in range(KT):
                            nc.tensor.matmul(
                                ps_q,
                                w_sb[:, k, ts(m, 128)],
                                x_sbT[:, k, ts(n, 512)],
                                start=(k == 0),
                                stop=(k == KT - 1),
                            )
                        for hh in range(2):
                            nc.vector.tensor_scalar_add(
                                dst[0:64, 2 * m + hh, ts(n, 512)],
                                ps_q[64 * hh : 64 * hh + 64, :],
                                b_sb[64 * hh : 64 * hh + 64, m : m + 1],
                            )

            emit_qk(0)

            v_sb = p_v.tile([128, TT, E], f32r, tag="vsb")
            v_bf = p_v.tile([128, TT, HPG * 65], bf16, tag="vbf")
            ones_ap = v_bf.rearrange("p i (h e) -> p i h e", h=HPG)[:, :, :, 64:65]
            nc.vector.memset(ones_ap, 1.0)
            for i in range(TT):
                ps_v = ps_mm.tile([128, 512], f32, tag="mm512")
                for k in range(KT):
                    nc.tensor.matmul(
                        ps_v[:, :E],
                        x_sbT[:, k, ts(i, 128)],
                        wv_sb[:, k, :],
                        start=(k == 0),
                        stop=(k == KT - 1),
                    )
                nc.vector.tensor_add(v_sb[:, i, :], ps_v[:, :E], bv_bc)
                # beta pre-scaled into the attv weights (ones col stays 1 so
                # psum row 64 remains the raw softmax denominator)
                nc.vector.tensor_scalar_mul(
                    v_bf[:, i, :].rearrange("p (h e) -> p h e", h=HPG)[:, :, 0:64],
                    v_sb[:, i, :],
                    beta_sb,
                )

            emit_qk(1)

            # ---- attention heads ----
            yT_all = p_y.tile([128, 2, T], f32r)
            for h in range(HPG):
                po = 64 * (h % 2)
                m = h // 2

                # cumsum of v rows (for the MC + Id terms), ahead of the
                # attention loop so PE transposes interleave with matmuls
                cum_sb = p_w.tile([64, T], f32, tag="wk")
                for i in range(TT):
                    ps_tr = ps_t.tile([64, 128], f32r, tag="tr")
                    nc.tensor.transpose(
                        ps_tr, v_sb[:, i, ds(64 * h, 64)], ident_sb
                    )
                    init = 0.0 if i == 0 else cum_sb[:, 128 * i - 1 : 128 * i]
                    nc.vector.tensor_tensor_scan(
                        cum_sb[:, ts(i, 128)], ps_tr.bitcast(f32), agt_bc[:, 0:128],
                        init, ALU.add, ALU.bypass,
                    )
                # va = cum*(alpha - gamma/(t+1)) - alpha*cum[t-1], computed
                # up front so cum_sb's slot frees before the next head
                # (va[0] = cum[0]*agt[0] already correct: cum[-1] := 0)
                va = p_pb.tile([64, T], f32, tag="t2")
                nc.vector.tensor_mul(va, cum_sb, agt_bc)
                nc.vector.scalar_tensor_tensor(
                    va[:, 1:T], cum_sb[:, 0 : T - 1], nalpha_sb[0:64, :],
                    va[:, 1:T], ALU.mult, ALU.add,
                )

                ps_yT = ps_y.tile([128, T], f32, tag="psy")
                for j in range(TT):
                    exp_sb = p_exp.tile([128, T], bf16, tag="exp")
                    for cb in range(j // 4, NB):
                        lo = max(512 * cb, 128 * j)
                        hi = 512 * (cb + 1)
                        w = hi - lo
                        ps_s = ps_mm.tile([128, 512], f32, tag="mm512")
                        diag = cb == j // 4
                        nc.tensor.matmul(
                            ps_s[:, :w],
                            kT_sb[:, h, ts(j, 128)],
                            qT_sb[:, h, ds(lo, w)],
                            start=True,
                            stop=not diag,
                        )
                        if diag:
                            # causal mask fused into the matmul: add
                            # -1e30 * [s > t] so exp underflows to zero
                            # above the diagonal
                            nc.tensor.matmul(
                                ps_s[:, 0:128], slt_sb, negid_sb,
                                start=False, stop=True,
                            )
                        nc.scalar.activation(
                            exp_sb[:, ds(lo - 128 * j, w)], ps_s[:, :w],
                            AFT.Exp, scale=0.125,
                        )
                        nc.tensor.matmul(
                            ps_yT[:65, ds(lo, w)],
                            v_bf[:, j, ds(65 * h, 65)],
                            exp_sb[:, ds(lo - 128 * j, w)],
                            start=(j == 0),
                            stop=(j == 4 * cb + 3),
                        )

                # free ps_yT quickly: pull the raw numerator and denominator
                # out of PSUM first so the next head's att@v can start.
                # 1/den via ACT Ln -> Exp(-x) (custom-DVE reciprocal ops are
                # broken on this toolchain; plain DVE reciprocal is ~6.5ns/elem)
                rden = p_pb.tile([1, T], f32, tag="rden")
                nc.scalar.activation(rden, ps_yT[64:65, :], AFT.Ln)
                t1 = p_w.tile([64, T], f32, tag="wv")
                for cb in range(NB):
                    nc.vector.tensor_copy(
                        t1[:, ts(cb, 512)], ps_yT[0:64, ts(cb, 512)]
                    )
                nc.scalar.activation(rden, rden, AFT.Exp, scale=-1.0)

                nc.sync.dma_start(bounce_d[h], rden)
                rb_bc = p_w.tile([64, T], f32, tag="wq")
                nc.sync.dma_start(
                    rb_bc,
                    bass.AP(tensor=bounce_d[h].tensor, offset=bounce_d[h].offset,
                            ap=[[0, 64]] + bounce_d[h].ap[1:]),
                )

                # yT = num*beta/den + va   (chunked so proj can start early)
                ydst = yT_all[po : po + 64, m, :]
                for cb in range(NB):
                    sl = ts(cb, 512)
                    nc.vector.tensor_mul(t1[:, sl], t1[:, sl], rb_bc[:, sl])
                    nc.vector.tensor_add(ydst[:, sl], t1[:, sl], va[:, sl])

            # ---- output projection (partial over this core's heads) ----
            out_sb = p_big.tile([128, KT, T], f32, tag="big")
            for n in range(NB):
                for o in range(KT):
                    ps_p = ps_mm.tile([128, 512], f32, tag="mm512")
                    for m in range(2):
                        nc.tensor.matmul(
                            ps_p,
                            wp_sb[:, m, ts(o, 128)],
                            yT_all[:, m, ts(n, 512)],
                            start=(m == 0),
                            stop=(m == 1),
                        )
                    nc.vector.tensor_copy(out_sb[:, o, ts(n, 512)], ps_p)
            for o in range(KT):
                nc.sync.dma_start(outT_d[ts(o, 128), :], out_sb[:, o, :])

    nc.compile()
    return nc


_NC_CACHE = None


def _get_nc():
    global _NC_CACHE
    if _NC_CACHE is None:
        _NC_CACHE = build_bass()
    return _NC_CACHE


def make_in_maps(x, W_attn, b_attn, W_proj, b_proj, alpha, beta, gamma):
    x = np.asarray(x, dtype=np.float32)
    W_attn = np.asarray(W_attn, dtype=np.float32)
    b_attn = np.asarray(b_attn, dtype=np.float32)
    W_proj = np.asarray(W_proj, dtype=np.float32)
    alpha = float(np.asarray(alpha))
    beta = float(np.asarray(beta))
    gamma = float(np.asarray(gamma))

    ident = np.eye(128, dtype=np.float32)
    slt = np.triu(np.ones((128, 128), dtype=np.float32), 1).astype(ml_dtypes.bfloat16)
    negid = (np.eye(128, dtype=np.float32) * -1e30).astype(ml_dtypes.bfloat16)
    agt = (alpha - gamma / np.arange(1, T + 1, dtype=np.float32)).reshape(1, T)
    nalpha = np.full((128, 1), -alpha, dtype=np.float32)
    betac = np.full((128, 1), beta, dtype=np.float32)

    in_maps = []
    for core in range(NCORES):
        b, g = divmod(core, G)
        sl = slice(E * g, E * (g + 1))
        in_maps.append({
            "xT": np.ascontiguousarray(x[b].T),
            "wq": np.ascontiguousarray(W_attn[sl, :].T),
            "wk": np.ascontiguousarray(W_attn[C:][sl, :].T),
            "wv": np.ascontiguousarray(W_attn[2 * C:][sl, :].T),
            "wp": np.ascontiguousarray(W_proj[:, sl].T),
            "bq": np.ascontiguousarray(b_attn[sl].reshape(2, 128).T),
            "bk": np.ascontiguousarray(b_attn[C:][sl].reshape(2, 128).T),
            "bv": np.ascontiguousarray(b_attn[2 * C:][sl].reshape(1, E)),
            "agt": agt,
            "nalpha": nalpha,
            "betac": betac,
            "ident": ident,
            "slt": slt,
            "negid": negid,
        })
    return in_maps


def _assemble(results, b_proj):
    b_proj = np.asarray(b_proj, dtype=np.float32)
    out = np.empty((B, T, C), dtype=np.float32)
    for b in range(B):
        acc = results[G * b]["outT"].copy()
        for g in range(1, G):
            acc += results[G * b + g]["outT"]
        out[b] = acc.T + b_proj
    return out


def kernel(x, W_attn, b_attn, W_proj, b_proj, alpha, beta, gamma):
    nc = _get_nc()
    in_maps = make_in_maps(x, W_attn, b_attn, W_proj, b_proj, alpha, beta, gamma)
    res = bass_utils.run_bass_kernel_spmd(nc, in_maps, core_ids=list(range(NCORES)))
    return _assemble(res.results, b_proj)


def run_profiled(inputs, trace_cores=None):
    """Run with NTFF hardware profiling; returns (output, BassKernelResults)."""
    nc = _get_nc()
    in_maps = make_in_maps(**inputs)
    res = bass_utils.run_bass_kernel_spmd(
        nc, in_maps, core_ids=list(range(NCORES)), trace=True,
        trace_cores=trace_cores,
    )
    return _assemble(res.results, inputs["b_proj"]), res


# revision 20
# speedup vs baseline: 1.1317x; 1.0612x over previous
"""Trainium2 Bass kernel for nn_CausalShapedAttention (B=2, T=2048, C=1024, H=16).

Sharding: 8 cores = 2 batches x 4 head-groups (4 heads each). Each core
computes qkv for its heads, causal shaped attention, and a partial output
projection; the host sums the 4 partial projections per batch and adds b_proj.

Math per head (d=64):
    scores = q @ k.T / 8           (only causal lower triangle computed)
    att    = exp(scores) / rowsum(exp(scores))     # no max-subtraction:
                                                   # |scores| <~ 4, exp safe
    y      = beta * att @ v + alpha * v - gamma * cummean(v)

On-device layouts (per core):
    xT  [C, T]    qT/kT [256, T] (bf16 for PE)     v [T, 256] (f32r)
    scoresT strips [s-block 128, t >= s]  -> exp (bf16) -> att@v accumulates
    yT_unnorm in PSUM [65, T] via ones-augmented beta*v (row 64 = softmax denom)
    cummean via PE transpose of v blocks + DVE prefix scan
    proj: outT [C, T] = wpT.T @ yT   (partial; host reduces over groups)
"""

import numpy as np
import ml_dtypes

import concourse.bass as bass
import concourse.mybir as mybir
import concourse.tile as tile
from concourse import bacc, bass_utils
from concourse.bass import ds, ts

B, T, C, H = 2, 2048, 1024, 16
NCORES = 8
G = 4                 # head-groups per batch
HPG = H // G          # heads per group = 4
D = C // H            # 64
E = HPG * D           # 256

f32 = mybir.dt.float32
f32r = mybir.dt.float32r
bf16 = mybir.dt.bfloat16
AFT = mybir.ActivationFunctionType
ALU = mybir.AluOpType

KT = C // 128         # 8 k-tiles over the embedding dim
TT = T // 128         # 16 t-tiles
NB = T // 512         # 4 psum banks over T


def build_bass():
    nc = bacc.Bacc("TRN2", target_bir_lowering=False, debug=False)

    # ---- DRAM I/O ----
    xT_d = nc.dram_tensor("xT", [C, T], f32r, kind="ExternalInput").ap()
    wq_d = nc.dram_tensor("wq", [C, E], f32r, kind="ExternalInput").ap()
    wk_d = nc.dram_tensor("wk", [C, E], f32r, kind="ExternalInput").ap()
    wv_d = nc.dram_tensor("wv", [C, E], f32r, kind="ExternalInput").ap()
    wp_d = nc.dram_tensor("wp", [E, C], f32r, kind="ExternalInput").ap()
    bq_d = nc.dram_tensor("bq", [128, 2], f32, kind="ExternalInput").ap()
    bk_d = nc.dram_tensor("bk", [128, 2], f32, kind="ExternalInput").ap()
    bv_d = nc.dram_tensor("bv", [1, E], f32, kind="ExternalInput").ap()
    agt_d = nc.dram_tensor("agt", [1, T], f32, kind="ExternalInput").ap()
    nalpha_d = nc.dram_tensor("nalpha", [128, 1], f32, kind="ExternalInput").ap()
    beta_d = nc.dram_tensor("betac", [128, 1], f32, kind="ExternalInput").ap()
    ident_d = nc.dram_tensor("ident", [128, 128], f32r, kind="ExternalInput").ap()
    slt_d = nc.dram_tensor("slt", [128, 128], bf16, kind="ExternalInput").ap()
    negid_d = nc.dram_tensor("negid", [128, 128], bf16, kind="ExternalInput").ap()
    outT_d = nc.dram_tensor("outT", [C, T], f32, kind="ExternalOutput").ap()

    bounce_d = [
        nc.dram_tensor(f"bounce{h}", [1, T], f32, kind="Internal").ap()
        for h in range(HPG)
    ]

    with tile.TileContext(nc) as tc:
        import contextlib

        with contextlib.ExitStack() as ctx:
            p_big = ctx.enter_context(tc.tile_pool(name="p_big", bufs=1))
            p_w = ctx.enter_context(tc.tile_pool(name="p_w", bufs=1))
            p_wp = ctx.enter_context(tc.tile_pool(name="p_wp", bufs=1))
            p_const = ctx.enter_context(tc.tile_pool(name="p_const", bufs=1))
            p_qk = ctx.enter_context(tc.tile_pool(name="p_qk", bufs=1))
            p_v = ctx.enter_context(tc.tile_pool(name="p_v", bufs=1))
            p_exp = ctx.enter_context(tc.tile_pool(name="p_exp", bufs=3))
            p_y = ctx.enter_context(tc.tile_pool(name="p_y", bufs=1))
            p_pb = ctx.enter_context(tc.tile_pool(name="p_pb", bufs=1))
            ps_mm = ctx.enter_context(tc.tile_pool(name="ps_mm", bufs=2, space="PSUM"))
            ps_y = ctx.enter_context(tc.tile_pool(name="ps_y", bufs=1, space="PSUM"))
            ps_t = ctx.enter_context(tc.tile_pool(name="ps_t", bufs=2, space="PSUM"))

            # ---- x streams in first (critical path), then weights ----
            x_sbT = p_big.tile([128, KT, T], f32r, tag="big")
            for k in range(KT):
                nc.sync.dma_start(x_sbT[:, k, :], xT_d[ts(k, 128), :])

            wq_sb = p_w.tile([128, KT, E], f32r, tag="wq")
            nc.sync.dma_start(wq_sb, wq_d.rearrange("(k p) j -> p k j", p=128))
            wk_sb = p_w.tile([128, KT, E], f32r, tag="wk")
            nc.sync.dma_start(wk_sb, wk_d.rearrange("(k p) j -> p k j", p=128))

            bq_sb = p_const.tile([128, 2], f32)
            nc.sync.dma_start(bq_sb, bq_d)
            bk_sb = p_const.tile([128, 2], f32)
            nc.sync.dma_start(bk_sb, bk_d)
            nalpha_sb = p_const.tile([128, 1], f32)
            nc.sync.dma_start(nalpha_sb, nalpha_d)
            beta_sb = p_const.tile([128, 1], f32)
            nc.sync.dma_start(beta_sb, beta_d)
            ident_sb = p_const.tile([128, 128], f32r)
            nc.sync.dma_start(ident_sb, ident_d)
            slt_sb = p_const.tile([128, 128], bf16)
            nc.sync.dma_start(slt_sb, slt_d)
            negid_sb = p_const.tile([128, 128], bf16)
            nc.sync.dma_start(negid_sb, negid_d)
            bv_bc = p_const.tile([128, E], f32)
            nc.sync.dma_start(
                bv_bc,
                bass.AP(tensor=bv_d.tensor, offset=bv_d.offset,
                        ap=[[0, 128]] + bv_d.ap[1:]),
            )
            agt_bc = p_const.tile([64, T], f32)
            nc.sync.dma_start(
                agt_bc,
                bass.AP(tensor=agt_d.tensor, offset=agt_d.offset,
                        ap=[[0, 64]] + agt_d.ap[1:]),
            )

            wv_sb = p_w.tile([128, KT, E], f32r, tag="wv")
            nc.sync.dma_start(wv_sb, wv_d.rearrange("(k p) j -> p k j", p=128))
            wp_sb = p_wp.tile([128, 2, C], f32r)
            nc.sync.dma_start(wp_sb, wp_d.rearrange("(m p) o -> p m o", p=128))

            # ---- qkv ----
            # m=0 (heads 0/1) first, then v, then m=1 — the m=1 matmuls act
            # as PE filler while head 0/1's ACT-bound softmax runs
            qT_sb = p_qk.tile([128, HPG, T], bf16, tag="qT")
            kT_sb = p_qk.tile([128, HPG, T], bf16, tag="kT")
            nc.vector.memset(qT_sb[64:128, :, :], 0.0)
            nc.vector.memset(kT_sb[64:128, :, :], 0.0)

            def emit_qk(m):
                for w_sb, b_sb, dst in (
                    (wq_sb, bq_sb, qT_sb),
                    (wk_sb, bk_sb, kT_sb),
                ):
                    for n in range(NB):
                        ps_q = ps_mm.tile([128, 512], f32, tag="mm512")
                        for k in range(KT):
                            nc.tensor.matmul(
                                ps_q,
                                w_sb[:, k, ts(m, 128)],
                                x_sbT[:, k, ts(n, 512)],
                                start=(k == 0),
                                stop=(k == KT - 1),
                            )
                        for hh in range(2):
                            nc.vector.tensor_scalar_add(
                                dst[0:64, 2 * m + hh, ts(n, 512)],
                                ps_q[64 * hh : 64 * hh + 64, :],
                                b_sb[64 * hh : 64 * hh + 64, m : m + 1],
                            )

            emit_qk(0)

            v_sb = p_v.tile([128, TT, E], f32r, tag="vsb")
            v_bf = p_v.tile([128, TT, HPG * 65], bf16, tag="vbf")
            ones_ap = v_bf.rearrange("p i (h e) -> p i h e", h=HPG)[:, :, :, 64:65]
            nc.vector.memset(ones_ap, 1.0)
            for i in range(TT):
                ps_v = ps_mm.tile([128, 512], f32, tag="mm512")
                for k in range(KT):
                    nc.tensor.matmul(
                        ps_v[:, :E],
                        x_sbT[:, k, ts(i, 128)],
                        wv_sb[:, k, :],
                        start=(k == 0),
                        stop=(k == KT - 1),
                    )
                nc.vector.tensor_add(v_sb[:, i, :], ps_v[:, :E], bv_bc)
                # beta pre-scaled into the attv weights (ones col stays 1 so
                # psum row 64 remains the raw softmax denominator)
                nc.vector.tensor_scalar_mul(
                    v_bf[:, i, :].rearrange("p (h e) -> p h e", h=HPG)[:, :, 0:64],
                    v_sb[:, i, :],
                    beta_sb,
                )

            emit_qk(1)

            # ---- attention heads ----
            yT_all = p_y.tile([128, 2, T], f32r)
            for h in range(HPG):
                po = 64 * (h % 2)
                m = h // 2

                # cumsum of v rows (for the MC + Id terms), ahead of the
                # attention loop so PE transposes interleave with matmuls
                cum_sb = p_w.tile([64, T], f32, tag="wk")
                for i in range(TT):
                    ps_tr = ps_t.tile([64, 128], f32r, tag="tr")
                    nc.tensor.transpose(
                        ps_tr, v_sb[:, i, ds(64 * h, 64)], ident_sb
                    )
                    init = 0.0 if i == 0 else cum_sb[:, 128 * i - 1 : 128 * i]
                    nc.vector.tensor_tensor_scan(
                        cum_sb[:, ts(i, 128)], ps_tr.bitcast(f32), agt_bc[:, 0:128],
                        init, ALU.add, ALU.bypass,
                    )
                # va = cum*(alpha - gamma/(t+1)) - alpha*cum[t-1], computed
                # up front so cum_sb's slot frees before the next head
                # (va[0] = cum[0]*agt[0] already correct: cum[-1] := 0)
                va = p_pb.tile([64, T], f32, tag="t2")
                nc.vector.tensor_mul(va, cum_sb, agt_bc)
                nc.vector.scalar_tensor_tensor(
                    va[:, 1:T], cum_sb[:, 0 : T - 1], nalpha_sb[0:64, :],
                    va[:, 1:T], ALU.mult, ALU.add,
                )

                ps_yT = ps_y.tile([128, T], f32, tag="psy")
                for j in range(TT):
                    exp_sb = p_exp.tile([128, T], bf16, tag="exp")
                    for cb in range(j // 4, NB):
                        lo = max(512 * cb, 128 * j)
                        hi = 512 * (cb + 1)
                        w = hi - lo
                        ps_s = ps_mm.tile([128, 512], f32, tag="mm512")
                        diag = cb == j // 4
                        nc.tensor.matmul(
                            ps_s[:, :w],
                            kT_sb[:, h, ts(j, 128)],
                            qT_sb[:, h, ds(lo, w)],
                            start=True,
                            stop=not diag,
                        )
                        if diag:
                            # causal mask fused into the matmul: add
                            # -1e30 * [s > t] so exp underflows to zero
                            # above the diagonal
                            nc.tensor.matmul(
                                ps_s[:, 0:128], slt_sb, negid_sb,
                                start=False, stop=True,
                            )
                        nc.scalar.activation(
                            exp_sb[:, ds(lo - 128 * j, w)], ps_s[:, :w],
                            AFT.Exp, scale=0.125,
                        )
                        nc.tensor.matmul(
                            ps_yT[:65, ds(lo, w)],
                            v_bf[:, j, ds(65 * h, 65)],
                            exp_sb[:, ds(lo - 128 * j, w)],
                            start=(j == 0),
                            stop=(j == 4 * cb + 3),
                        )

                # free ps_yT quickly: pull the raw numerator and denominator
                # out of PSUM first so the next head's att@v can start.
                # 1/den via ACT Ln -> Exp(-x) (custom-DVE reciprocal ops are
                # broken on this toolchain; plain DVE reciprocal is ~6.5ns/elem)
                rden = p_pb.tile([1, T], f32, tag="rden")
                nc.scalar.activation(rden, ps_yT[64:65, :], AFT.Ln)
                t1 = p_w.tile([64, T], f32, tag="wv")
                for cb in range(NB):
                    nc.vector.tensor_copy(
                        t1[:, ts(cb, 512)], ps_yT[0:64, ts(cb, 512)]
                    )
                nc.scalar.activation(rden, rden, AFT.Exp, scale=-1.0)

                nc.sync.dma_start(bounce_d[h], rden)
                rb_bc = p_w.tile([64, T], f32, tag="wq")
                nc.sync.dma_start(
                    rb_bc,
                    bass.AP(tensor=bounce_d[h].tensor, offset=bounce_d[h].offset,
                            ap=[[0, 64]] + bounce_d[h].ap[1:]),
                )

                # yT = num*beta/den + va   (chunked so proj can start early)
                ydst = yT_all[po : po + 64, m, :]
                for cb in range(NB):
                    sl = ts(cb, 512)
                    nc.vector.tensor_mul(t1[:, sl], t1[:, sl], rb_bc[:, sl])
                    nc.vector.tensor_add(ydst[:, sl], t1[:, sl], va[:, sl])

            # ---- output projection (partial over this core's heads) ----
            out_sb = p_big.tile([128, KT, T], f32, tag="big")
            for o in range(KT):
                for n in range(NB):
                    ps_p = ps_mm.tile([128, 512], f32, tag="mm512")
                    for m in range(2):
                        nc.tensor.matmul(
                            ps_p,
                            wp_sb[:, m, ts(o, 128)],
                            yT_all[:, m, ts(n, 512)],
                            start=(m == 0),
                            stop=(m == 1),
                        )
                    nc.vector.tensor_copy(out_sb[:, o, ts(n, 512)], ps_p)
                nc.sync.dma_start(outT_d[ts(o, 128), :], out_sb[:, o, :])

    nc.compile()
    return nc


_NC_CACHE = None


def _get_nc():
    global _NC_CACHE
    if _NC_CACHE is None:
        _NC_CACHE = build_bass()
    return _NC_CACHE


def make_in_maps(x, W_attn, b_attn, W_proj, b_proj, alpha, beta, gamma):
    x = np.asarray(x, dtype=np.float32)
    W_attn = np.asarray(W_attn, dtype=np.float32)
    b_attn = np.asarray(b_attn, dtype=np.float32)
    W_proj = np.asarray(W_proj, dtype=np.float32)
    alpha = float(np.asarray(alpha))
    beta = float(np.asarray(beta))
    gamma = float(np.asarray(gamma))

    ident = np.eye(128, dtype=np.float32)
    slt = np.triu(np.ones((128, 128), dtype=np.float32), 1).astype(ml_dtypes.bfloat16)
    negid = (np.eye(128, dtype=np.float32) * -1e30).astype(ml_dtypes.bfloat16)
    agt = (alpha - gamma / np.arange(1, T + 1, dtype=np.float32)).reshape(1, T)
    nalpha = np.full((128, 1), -alpha, dtype=np.float32)
    betac = np.full((128, 1), beta, dtype=np.float32)

    in_maps = []
    for core in range(NCORES):
        b, g = divmod(core, G)
        sl = slice(E * g, E * (g + 1))
        in_maps.append({
            "xT": np.ascontiguousarray(x[b].T),
            "wq": np.ascontiguousarray(W_attn[sl, :].T),
            "wk": np.ascontiguousarray(W_attn[C:][sl, :].T),
            "wv": np.ascontiguousarray(W_attn[2 * C:][sl, :].T),
            "wp": np.ascontiguousarray(W_proj[:, sl].T),
            "bq": np.ascontiguousarray(b_attn[sl].reshape(2, 128).T),
            "bk": np.ascontiguousarray(b_attn[C:][sl].reshape(2, 128).T),
            "bv": np.ascontiguousarray(b_attn[2 * C:][sl].reshape(1, E)),
            "agt": agt,
            "nalpha": nalpha,
            "betac": betac,
            "ident": ident,
            "slt": slt,
            "negid": negid,
        })
    return in_maps


def _assemble(results, b_proj):
    b_proj = np.asarray(b_proj, dtype=np.float32)
    out = np.empty((B, T, C), dtype=np.float32)
    for b in range(B):
        acc = results[G * b]["outT"].copy()
        for g in range(1, G):
            acc += results[G * b + g]["outT"]
        out[b] = acc.T + b_proj
    return out


def kernel(x, W_attn, b_attn, W_proj, b_proj, alpha, beta, gamma):
    nc = _get_nc()
    in_maps = make_in_maps(x, W_attn, b_attn, W_proj, b_proj, alpha, beta, gamma)
    res = bass_utils.run_bass_kernel_spmd(nc, in_maps, core_ids=list(range(NCORES)))
    return _assemble(res.results, b_proj)


def run_profiled(inputs, trace_cores=None):
    """Run with NTFF hardware profiling; returns (output, BassKernelResults)."""
    nc = _get_nc()
    in_maps = make_in_maps(**inputs)
    res = bass_utils.run_bass_kernel_spmd(
        nc, in_maps, core_ids=list(range(NCORES)), trace=True,
        trace_cores=trace_cores,
    )
    return _assemble(res.results, inputs["b_proj"]), res
